# revision 31
# baseline (speedup 1.0000x reference)
"""Trainium2 Bass kernel for top-2 MoE (nn_MoE_2113123910117).

Strategy (expert-parallel + combine-weight-aware mixed precision):
  - Host: router softmax -> top-2 -> normalized combine weights. Every
    top-1 (token, expert) pair and every top-2 pair with combine weight
    s2 >= TAU runs on device in fp16. Top-2 pairs with s2 < TAU (~52% of
    them) run on device in fp8e4m3 using DoubleRow matmuls (2x PE MAC
    rate); their output error (~6.6% rel) is attenuated by s2 < 0.38,
    keeping the global rel error ~1.6e-2 (< 2e-2 budget).
  - Cores are grouped in pairs; each pair owns two experts. Per core the
    NEFF runs 4 sub-batches: A1/A2 = fp16 FFN over 768 tokens for expert
    i/j, B1/B2 = fp8 FFN over 256 tokens for expert i/j. Each expert's
    fp16 tokens split across its pair's two A-slots (2x768 = 1536 cap),
    fp8 tokens across the B-slots (512 cap). Capacity overflow (~1.5% of
    pairs, chosen as the largest-s2 fp8 tokens) is computed on host in
    fp32 and added during unshard.
  - fp16 path: y = diag(s) ((silu(x Wg^T) * (x Wu^T)) Wd^T), fp16 matmul
    operands, fp32 PSUM.
  - fp8 path: weights pre-scaled by 8 and quantized to fp8e4m3; x
    quantized to fp8e4m3. DoubleRow matmuls with K=256 per instruction.
    silu applied with scale 1/8 on ACT (fp16 out), h quantized to fp8 by
    the DVE multiply, down-projection epilogue scale s/64 folds out the
    weight prescaling.

Self-contained: hardcodes all shapes from the problem spec.
"""

import os
import numpy as np

# recover automatically if a prior run left the NeuronCores wedged
os.environ.setdefault("NEURON_RT_RESET_CORES", "1")

D = 1024
FF = 2048
E = 8
TOPK = 2
NCORES = 8
ND = D // 128    # 8 contraction chunks of 128
NF = FF // 128   # 16 ff chunks of 128
NKP = ND // 2    # 4 k-pairs (256-deep DoubleRow contraction) for gate/up
NFP = NF // 2    # 8 k-pairs for the fp8 down projection
CAP16 = 1536     # fp16 tokens per core (2 sub-batches of 768)
CAP8 = 512       # fp8 tokens per core (2 sub-batches of 256)
ATT = 256        # fp16 token tile
TAU = 0.38       # top-2 combine-weight threshold for the fp8 path
SW = 8.0         # fp8 weight prescale; down epilogue folds 1/SW^2

# test-only knobs / results (harness never touches these)
LAST_RESULTS = None
_NC_CACHE = {}


def split_multi_waits(nc, mybir_mod):
    """This walrus build rejects any instruction carrying more than one
    sync wait ("Too many sync wait commands"). Hoist extra waits onto
    single-wait NOPs inserted just before the instruction on the same
    engine — semantically identical since engines execute in order."""
    n_split = 0
    for f in nc.m.functions:
        for blk in f.blocks:
            insts = blk.instructions
            newl = []
            changed = False
            for inst in insts:
                si = inst.sync_info
                if si is not None and len(si.on_wait) > 1:
                    waits = list(si.on_wait)
                    del si.on_wait[1:]
                    for j, w in enumerate(waits[1:]):
                        nop = mybir_mod.InstNoOp(
                            name=f"{inst.name}_w{j}",
                            engine=inst.engine,
                            ins=[],
                            outs=[],
                        )
                        nop.sync_info = mybir_mod.SyncInfo(on_wait=[w], on_update=[])
                        newl.append(nop)
                        n_split += 1
                    changed = True
                newl.append(inst)
            if changed:
                insts[:] = newl
    return n_split


def build_nc(caps=(CAP16, CAP8), repeat=1):
    """Build the per-core Bass program: two fp16 expert FFN sub-batches
    followed by two fp8 (DoubleRow) sub-batches. Same NEFF on all 8
    cores (SPMD); the expert identity lives entirely in the input maps.

    repeat>1 wraps the whole body (including weight loads) in a hardware
    loop — used only for benchmarking (dispatch overhead amortization)."""
    import contextlib

    import concourse.bass as bass
    import concourse.mybir as mybir
    import concourse.tile as tile

    dt = mybir.dt
    f32 = dt.float32
    f16 = dt.float16
    e4 = dt.float8e4
    AF = mybir.ActivationFunctionType
    DR = mybir.MatmulPerfMode.DoubleRow

    cap16, cap8 = caps
    a_sub = cap16 // 2
    b_sub = cap8 // 2
    na_t = a_sub // ATT       # fp16 token tiles per sub-batch
    ng16 = cap16 // 128       # combine-scale groups, fp16
    ng8 = cap8 // 128

    nc = bass.Bass()
    # fp16 tokens: x^T arranged [128, d-chunk, token]
    xt = nc.dram_tensor("xt", [128, ND, cap16], f16, kind="ExternalInput")
    # fp8 tokens, same layout
    x8 = nc.dram_tensor("x8", [128, ND, cap8], e4, kind="ExternalInput")
    # fp16 weights, one set per expert of the pair; gate/up f-chunk-major
    wg = nc.dram_tensor("wg", [2, NF * 128, ND, 128], f16, kind="ExternalInput")
    wu = nc.dram_tensor("wu", [2, NF * 128, ND, 128], f16, kind="ExternalInput")
    wd = nc.dram_tensor("wd", [2, FF, D], f16, kind="ExternalInput")
    # fp8 weights (pre-scaled by SW): [set, chunk*128(+p), kpair, ktile, feat]
    wg8 = nc.dram_tensor("wg8", [2, NF * 128, NKP, 2, 128], e4, kind="ExternalInput")
    wu8 = nc.dram_tensor("wu8", [2, NF * 128, NKP, 2, 128], e4, kind="ExternalInput")
    # fp8 down weights: [set, p(f within chunk), kpair, ktile, d]
    wd8 = nc.dram_tensor("wd8", [2, 128, NFP, 2, D], e4, kind="ExternalInput")
    sc = nc.dram_tensor("sc", [128, ng16], f32, kind="ExternalInput")
    sc8 = nc.dram_tensor("sc8", [128, ng8], f32, kind="ExternalInput")
    # outputs at fp16: partial rows are combined on host in fp32
    y = nc.dram_tensor("y", [cap16, D], f16, kind="ExternalOutput")
    y8 = nc.dram_tensor("y8", [cap8, D], f16, kind="ExternalOutput")

    with tile.TileContext(nc) as tc:
        with (
            tc.tile_pool(name="wpool", bufs=1) as wpool,
            tc.tile_pool(name="w8pool", bufs=1) as w8pool,
            tc.tile_pool(name="xpool", bufs=3) as xpool,
            tc.tile_pool(name="hpool", bufs=2) as hpool,
            tc.tile_pool(name="gpool", bufs=3) as gpool,
            tc.tile_pool(name="g8pool", bufs=2) as g8pool,
            tc.tile_pool(name="ypool", bufs=4) as ypool,
            tc.tile_pool(name="pg", bufs=2, space="PSUM") as pgpool,
            tc.tile_pool(name="pu", bufs=2, space="PSUM") as pupool,
            tc.tile_pool(name="po", bufs=4, space="PSUM") as popool,
        ):
            # PE warmup: matmuls on a zeroed scratch tile while the first
            # DMAs are in flight, so the p-state clock gate is already
            # ramping when real matmuls start. Outside the repeat loop.
            warm = wpool.tile([128, 128], f16, tag="warm")
            nc.gpsimd.memset(warm[:], 0)
            pwarm = pgpool.tile([128, 512], f32, tag="pg")
            for i in range(44):
                nc.tensor.matmul(pwarm[:, 0:128], warm[:], warm[:])
            rep_ctx = (
                tc.For_i(0, repeat, 1, hint_engines=(mybir.EngineType.PE,))
                if repeat > 1
                else contextlib.nullcontext()
            )
            rep_ctx.__enter__()

            # Resident fp8 token tensor; fp16 tokens stream in 256-tiles.
            x8_sb = wpool.tile([128, ND, cap8], e4, tag="x8")
            s_sb = wpool.tile([128, ng16], f32, tag="s")
            s8_sb = wpool.tile([128, ng8], f32, tag="s8")

            # ---------------- phase A: fp16, two sub-batches ----------------
            for s in range(2):
                # DMA issue order == compute-need order. First sub-batch:
                # token tiles interleaved with the first weight slices so
                # the first matmul waits only ~1MB of DMA.
                wg_sb = [None] * NF
                wu_sb = [None] * NF
                xt_t = [None] * na_t

                def load_xt(t):
                    off = s * a_sub + t * ATT
                    xti = xpool.tile([128, ND, ATT], f16, tag="xt")
                    nc.sync.dma_start(xti[:], xt[:, :, off : off + ATT])
                    xt_t[t] = xti

                for f in range(NF):
                    if f == 0:
                        load_xt(0)
                    tg = wpool.tile([128, ND, 128], f16, tag=f"wg{f}")
                    nc.sync.dma_start(tg[:], wg[s, f * 128 : (f + 1) * 128])
                    wg_sb[f] = tg
                    tu = wpool.tile([128, ND, 128], f16, tag=f"wu{f}")
                    nc.sync.dma_start(tu[:], wu[s, f * 128 : (f + 1) * 128])
                    wu_sb[f] = tu
                    # token tiles interleaved into the weight stream in
                    # compute-need order (tile t is consumed only after all
                    # 16 f-chunks of tile t-1)
                    if f == 7 and na_t > 1:
                        load_xt(1)
                    elif f == NF - 1:
                        if s == 0:
                            nc.sync.dma_start(s_sb[:], sc[:])
                        for t in range(2, na_t):
                            load_xt(t)
                wd_sb = []
                for f in range(NF):
                    t_ = wpool.tile([128, D], f16, tag=f"wd{f}")
                    nc.sync.dma_start(t_[:], wd[s, f * 128 : (f + 1) * 128, :])
                    wd_sb.append(t_)

                for t in range(na_t):
                    off = s * a_sub + t * ATT
                    # gate/up + SwiGLU -> h^T [f, tokens]
                    ht_t = []
                    for f in range(NF):
                        pg = pgpool.tile([128, 512], f32, tag="pg")
                        pu = pupool.tile([128, 512], f32, tag="pu")
                        for j in range(ND):
                            nc.tensor.matmul(
                                pg[:, 0:ATT],
                                wg_sb[f][:, j, :],
                                xt_t[t][:, j, :],
                                start=(j == 0),
                                stop=(j == ND - 1),
                            )
                        for j in range(ND):
                            nc.tensor.matmul(
                                pu[:, 0:ATT],
                                wu_sb[f][:, j, :],
                                xt_t[t][:, j, :],
                                start=(j == 0),
                                stop=(j == ND - 1),
                            )
                        sg = gpool.tile([128, ATT], f16, tag="sg")
                        nc.scalar.activation(sg[:], pg[:, 0:ATT], AF.Silu)
                        ht = hpool.tile([128, ATT], f16, tag=f"ht{f}")
                        nc.vector.tensor_mul(ht[:], sg[:], pu[:, 0:ATT])
                        ht_t.append(ht)
                    # down projection, scaled by combine weight per token
                    for k in range(ATT // 128):
                        g = off // 128 + k
                        po_h = []
                        for dh in range(2):
                            po = popool.tile(
                                [128, 512], f32, tag="po", name=f"po_{s}_{t}_{k}_{dh}"
                            )
                            po_h.append(po)
                        for f in range(NF):
                            lhs = ht_t[f][:, k * 128 : (k + 1) * 128]
                            for dh in range(2):
                                nc.tensor.matmul(
                                    po_h[dh][:],
                                    lhs,
                                    wd_sb[f][:, dh * 512 : (dh + 1) * 512],
                                    start=(f == 0),
                                    stop=(f == NF - 1),
                                )
                        # scale-by-combine-weight copies: dh=0 on ACT, dh=1
                        # on DVE so the two run concurrently
                        for dh in range(2):
                            yt = ypool.tile([128, 512], f16, tag="yt")
                            if dh == 0:
                                nc.scalar.activation(
                                    yt[:], po_h[dh][:], AF.Copy,
                                    scale=s_sb[:, g : g + 1],
                                )
                            else:
                                nc.vector.tensor_scalar_mul(
                                    yt[:], po_h[dh][:], s_sb[:, g : g + 1]
                                )
                            dge = nc.sync if dh == 0 else nc.scalar
                            dge.dma_start(
                                y[
                                    g * 128 : (g + 1) * 128,
                                    dh * 512 : (dh + 1) * 512,
                                ],
                                yt[:],
                            )

            # ---------------- phase B: fp8 DoubleRow, two sub-batches -------
            # All phase-B DMAs are issued up front (they queue behind phase
            # A's loads and run during phase A compute). Gate/up set-1 tags
            # release chunk-by-chunk as sub-batch 0 consumes them; wd8 gets
            # one tag per set so set 1 streams during sub-batch 0 compute.
            wg8_sb = [{}, {}]
            wu8_sb = [{}, {}]
            wd8_sb = [None, None]
            nc.sync.dma_start(x8_sb[:], x8[:])
            nc.sync.dma_start(s8_sb[:], sc8[:])
            for c in range(NF):
                t8g = w8pool.tile([128, NKP, 2, 128], e4, tag=f"wg8_{c}")
                nc.sync.dma_start(t8g[:], wg8[0, c * 128 : (c + 1) * 128])
                wg8_sb[0][c] = t8g
                t8u = w8pool.tile([128, NKP, 2, 128], e4, tag=f"wu8_{c}")
                nc.sync.dma_start(t8u[:], wu8[0, c * 128 : (c + 1) * 128])
                wu8_sb[0][c] = t8u
            t8d = w8pool.tile([128, NFP, 2, D], e4, tag="wd8_0")
            nc.sync.dma_start(t8d[:], wd8[0])
            wd8_sb[0] = t8d

            for s in range(2):
                boff = s * b_sub
                ht8 = {}
                for cp in range(NF // 2):
                    # two 128-feature chunks share one [128,512] psum bank:
                    # only the chain's first matmul uses start=True — it
                    # zero-marks the whole 2KB bank, so the second chunk's
                    # chain accumulates onto pending-zero bytes
                    pg = pgpool.tile([128, 512], f32, tag="pg")
                    pu = pupool.tile([128, 512], f32, tag="pu")
                    for q in range(2):
                        c = 2 * cp + q
                        for j in range(NKP):
                            nc.tensor.matmul(
                                pg[:, q * ATT : (q + 1) * ATT],
                                wg8_sb[s][c][:, j, :, :],
                                x8_sb[:, 2 * j : 2 * j + 2, boff : boff + b_sub],
                                start=(q == 0 and j == 0),
                                stop=(q == 1 and j == NKP - 1),
                                perf_mode=DR,
                                skip_group_check=True,
                            )
                    for q in range(2):
                        c = 2 * cp + q
                        for j in range(NKP):
                            nc.tensor.matmul(
                                pu[:, q * ATT : (q + 1) * ATT],
                                wu8_sb[s][c][:, j, :, :],
                                x8_sb[:, 2 * j : 2 * j + 2, boff : boff + b_sub],
                                start=(q == 0 and j == 0),
                                stop=(q == 1 and j == NKP - 1),
                                perf_mode=DR,
                                skip_group_check=True,
                            )
                    sg = g8pool.tile([128, 512], f16, tag="sg8")
                    nc.scalar.activation(sg[:], pg[:], AF.Silu, scale=1.0 / SW)
                    hj = hpool.tile([128, 2, b_sub], e4, tag=f"ht8_{cp}")
                    ht8[cp] = hj
                    nc.vector.tensor_mul(hj[:], sg[:], pu[:])
                if s == 0:
                    # set-1 weight DMAs: emitted after sub-batch 0's gate/up
                    # reads are traced (tag reuse is then safe) so they
                    # stream during sub-batch 0's down sweep
                    for c in range(NF):
                        t8g = w8pool.tile([128, NKP, 2, 128], e4, tag=f"wg8_{c}")
                        nc.sync.dma_start(t8g[:], wg8[1, c * 128 : (c + 1) * 128])
                        wg8_sb[1][c] = t8g
                        t8u = w8pool.tile([128, NKP, 2, 128], e4, tag=f"wu8_{c}")
                        nc.sync.dma_start(t8u[:], wu8[1, c * 128 : (c + 1) * 128])
                        wu8_sb[1][c] = t8u
                    t8d = w8pool.tile([128, NFP, 2, D], e4, tag="wd8_1")
                    nc.sync.dma_start(t8d[:], wd8[1])
                    wd8_sb[1] = t8d
                # down projection: token groups of 128, d-halves of 512; the
                # two 256-wide chains of a half share one psum bank (start
                # only on the first) and one stationary load per j
                for k in range(b_sub // 128):
                    g = s * (b_sub // 128) + k
                    for dh in range(2):
                        po = popool.tile(
                            [128, 512], f32, tag="po", name=f"po8_{s}_{k}_{dh}"
                        )
                        for j in range(NFP):
                            for q in range(2):
                                nc.tensor.matmul(
                                    po[:, q * 256 : (q + 1) * 256],
                                    ht8[j][:, :, k * 128 : (k + 1) * 128],
                                    wd8_sb[s][
                                        :, j, :,
                                        dh * 512 + q * 256 : dh * 512 + (q + 1) * 256,
                                    ],
                                    start=(j == 0 and q == 0),
                                    stop=(j == NFP - 1 and q == 1),
                                    perf_mode=DR,
                                    skip_group_check=True,
                                )
                        yt = ypool.tile([128, 512], f16, tag="yt")
                        if dh == 0:
                            nc.scalar.activation(
                                yt[:], po[:], AF.Copy, scale=s8_sb[:, g : g + 1]
                            )
                        else:
                            nc.vector.tensor_scalar_mul(
                                yt[:], po[:], s8_sb[:, g : g + 1]
                            )
                        dge = nc.sync if dh == 0 else nc.scalar
                        dge.dma_start(
                            y8[
                                g * 128 : (g + 1) * 128,
                                dh * 512 : (dh + 1) * 512,
                            ],
                            yt[:],
                        )
            rep_ctx.__exit__(None, None, None)
    split_multi_waits(nc, mybir)
    return nc


def _get_nc(caps=(CAP16, CAP8)):
    key = caps
    if key not in _NC_CACHE:
        _NC_CACHE[key] = build_nc(caps)
    return _NC_CACHE[key]


def _route(xf, Wr):
    """fp32 softmax + top-2 + normalized combine weights, matching the
    jax reference (ties broken toward lower expert index)."""
    logits = xf @ Wr.astype(np.float32).T
    m = logits.max(-1, keepdims=True)
    ex = np.exp(logits - m)
    p = ex / ex.sum(-1, keepdims=True)
    top2 = np.argsort(-p, axis=-1, kind="stable")[:, :TOPK]
    n = xf.shape[0]
    p1 = p[np.arange(n), top2[:, 0]]
    p2 = p[np.arange(n), top2[:, 1]]
    denom = (p1 + p2) + np.float32(1e-8)
    return top2, p1 / denom, p2 / denom


def _pack_w16(W, mmnp):
    """Gate/up fp16 weight packing: [NF*128, ND, 128] f-chunk-major."""
    WT = W.T.reshape(ND, 128, NF, 128)
    return np.ascontiguousarray(
        WT.transpose(2, 1, 0, 3).reshape(NF * 128, ND, 128)
    ).astype(mmnp)


def _pack_w8_gu(W, e4np):
    """fp8 gate/up packing: [NF*128, NKP, 2, 128] where
    [c*128+p, j, i, m] = SW*W[c*128+m, (2j+i)*128+p]."""
    t = (W * SW).reshape(NF, 128, NKP, 2, 128)  # [c, m, j, i, p]
    return np.ascontiguousarray(t.transpose(0, 4, 2, 3, 1)).reshape(
        NF * 128, NKP, 2, 128
    ).astype(e4np)


def _pack_w8_d(Wde, e4np):
    """fp8 down packing: [128, NFP, 2, D] where
    [p, j, i, d] = SW*Wd[d, (2j+i)*128+p]."""
    t = (Wde * SW).reshape(D, NFP, 2, 128)  # [d, j, i, p]
    return np.ascontiguousarray(t.transpose(3, 1, 2, 0)).astype(e4np)


def _prep_maps(inputs, caps=(CAP16, CAP8)):
    """Route + build per-core input maps. Returns
    (in_maps, seg16, seg8, overflow, xf) where seg16/seg8 map core ->
    token index array for the y/y8 outputs and overflow is a list of
    (expert, token_idx_array, scale_array) computed on host."""
    import ml_dtypes

    cap16, cap8 = caps
    a_sub = cap16 // 2
    b_sub = cap8 // 2
    x = np.asarray(inputs["x"])
    Wr = np.asarray(inputs["Wr"])
    Wg = np.asarray(inputs["Wg"])
    Wu = np.asarray(inputs["Wu"])
    Wd = np.asarray(inputs["Wd"])
    xf = x.reshape(-1, D).astype(np.float32, copy=False)

    top2, s1, s2 = _route(xf, Wr)

    f16np = np.dtype(np.float16)
    e4np = np.dtype(ml_dtypes.float8_e4m3)
    xf_16 = xf.astype(f16np)
    xf_8 = xf.astype(e4np)

    # per-expert token lists
    idx16_e, sc16_e, idx8_e, sc8_e = [], [], [], []
    overflow = []
    n = xf.shape[0]
    for e in range(E):
        i_top1 = np.nonzero(top2[:, 0] == e)[0]
        s_top1 = s1[i_top1]
        i_top2 = np.nonzero(top2[:, 1] == e)[0]
        s_top2 = s2[i_top2]
        hi = s_top2 >= TAU
        i16 = np.concatenate([i_top1, i_top2[hi]])
        sc16v = np.concatenate([s_top1, s_top2[hi]]).astype(np.float32)
        # fp8 candidates sorted ascending s2 so capacity spill takes the
        # largest-s2 tokens (computed exactly on host -> least fp8 error)
        i8 = i_top2[~hi]
        s8v = s_top2[~hi].astype(np.float32)
        o = np.argsort(s8v, kind="stable")
        i8, s8v = i8[o], s8v[o]
        if len(i16) > cap16:
            overflow.append((e, i16[cap16:], sc16v[cap16:]))
            i16, sc16v = i16[:cap16], sc16v[:cap16]
        if len(i8) > cap8:
            overflow.append((e, i8[cap8:], s8v[cap8:]))
            i8, s8v = i8[:cap8], s8v[:cap8]
        idx16_e.append(i16)
        sc16_e.append(sc16v)
        idx8_e.append(i8)
        sc8_e.append(s8v)

    # pair experts to balance per-core cycles (384/pair fp16, 192 fp8)
    cyc = [384 * len(idx16_e[e]) + 192 * len(idx8_e[e]) for e in range(E)]
    order = np.argsort(np.asarray(cyc))
    pairs = [(int(order[7 - k]), int(order[k])) for k in range(E // 2)]

    in_maps = [None] * NCORES
    seg16 = [None] * NCORES
    seg8 = [None] * NCORES
    for k, (ea, eb) in enumerate(pairs):
        wgs = np.stack([_pack_w16(Wg[ea], f16np), _pack_w16(Wg[eb], f16np)])
        wus = np.stack([_pack_w16(Wu[ea], f16np), _pack_w16(Wu[eb], f16np)])
        wds = np.stack(
            [
                np.ascontiguousarray(Wd[ea].T).astype(f16np),
                np.ascontiguousarray(Wd[eb].T).astype(f16np),
            ]
        )
        wg8s = np.stack([_pack_w8_gu(Wg[ea], e4np), _pack_w8_gu(Wg[eb], e4np)])
        wu8s = np.stack([_pack_w8_gu(Wu[ea], e4np), _pack_w8_gu(Wu[eb], e4np)])
        wd8s = np.stack([_pack_w8_d(Wd[ea], e4np), _pack_w8_d(Wd[eb], e4np)])
        for half in range(2):
            core = 2 * k + half
            xt3 = np.zeros((128, ND, cap16), dtype=f16np)
            x83 = np.zeros((128, ND, cap8), dtype=e4np)
            scp = np.zeros(cap16, dtype=np.float32)
            sc8p = np.zeros(cap8, dtype=np.float32)
            t16 = []
            t8 = []
            for s, e_ in enumerate((ea, eb)):
                i16, v16 = idx16_e[e_], sc16_e[e_]
                nhalf = (len(i16) + 1) // 2
                seg = i16[:nhalf] if half == 0 else i16[nhalf:]
                vseg = v16[:nhalf] if half == 0 else v16[nhalf:]
                assert len(seg) <= a_sub
                off = s * a_sub
                xt3[:, :, off : off + len(seg)] = (
                    xf_16[seg].T.reshape(ND, 128, len(seg)).transpose(1, 0, 2)
                )
                scp[off : off + len(seg)] = vseg
                t16.append(seg)
                i8, v8 = idx8_e[e_], sc8_e[e_]
                nhalf8 = (len(i8) + 1) // 2
                seg8_ = i8[:nhalf8] if half == 0 else i8[nhalf8:]
                vseg8 = v8[:nhalf8] if half == 0 else v8[nhalf8:]
                assert len(seg8_) <= b_sub
                off8 = s * b_sub
                x83[:, :, off8 : off8 + len(seg8_)] = (
                    xf_8[seg8_]
                    .astype(e4np)
                    .T.reshape(ND, 128, len(seg8_))
                    .transpose(1, 0, 2)
                )
                sc8p[off8 : off8 + len(seg8_)] = vseg8 / np.float32(SW * SW)
                t8.append(seg8_)
            in_maps[core] = {
                "xt": xt3,
                "x8": x83,
                "wg": wgs,
                "wu": wus,
                "wd": wds,
                "wg8": wg8s,
                "wu8": wu8s,
                "wd8": wd8s,
                "sc": np.ascontiguousarray(
                    scp.reshape(cap16 // 128, 128).T
                ),
                "sc8": np.ascontiguousarray(
                    sc8p.reshape(cap8 // 128, 128).T
                ),
            }
            seg16[core] = t16
            seg8[core] = t8
    return in_maps, seg16, seg8, overflow, xf


def kernel(**inputs):
    global LAST_RESULTS
    from concourse.bass_utils import run_bass_kernel_spmd

    x = np.asarray(inputs["x"])
    B, T, _ = x.shape
    caps = (CAP16, CAP8)
    in_maps, seg16, seg8, overflow, xf = _prep_maps(inputs, caps)
    n_tok = xf.shape[0]
    a_sub, b_sub = caps[0] // 2, caps[1] // 2

    nc = _get_nc(caps)
    try:
        res = run_bass_kernel_spmd(nc, in_maps, list(range(NCORES)))
    except Exception:
        # A previously wedged NeuronCore fails the first execute attempt
        # (NRT_EXEC_UNIT_UNRECOVERABLE); resetting the PJRT backend and
        # retrying once recovers (cores reset via NEURON_RT_RESET_CORES).
        import jax
        import jax.extend as jex

        jax.clear_caches()
        try:
            jex.backend.clear_backends()
        except Exception:
            pass
        res = run_bass_kernel_spmd(nc, in_maps, list(range(NCORES)))
    LAST_RESULTS = res

    out = np.zeros((n_tok, D), dtype=np.float32)
    for core in range(NCORES):
        y16 = np.asarray(res.results[core]["y"]).astype(np.float32)
        y8v = np.asarray(res.results[core]["y8"]).astype(np.float32)
        for s in range(2):
            seg = seg16[core][s]
            out[seg] += y16[s * a_sub : s * a_sub + len(seg)]
            sg8 = seg8[core][s]
            out[sg8] += y8v[s * b_sub : s * b_sub + len(sg8)]
    # host-side capacity-overflow spill (fp32, exact)
    if overflow:
        Wg = np.asarray(inputs["Wg"], dtype=np.float32)
        Wu = np.asarray(inputs["Wu"], dtype=np.float32)
        Wd = np.asarray(inputs["Wd"], dtype=np.float32)
        for e, oidx, osc in overflow:
            xo = xf[oidx]
            g = xo @ Wg[e].T
            u = xo @ Wu[e].T
            h = (g / (1.0 + np.exp(-g))) * u
            out[oidx] += osc[:, None] * (h @ Wd[e].T)
    return out.reshape(B, T, D).astype(x.dtype, copy=False)


# revision 35
# speedup vs baseline: 2.9451x; 2.9451x over previous
"""Trainium2 Bass kernel for top-2 MoE (nn_MoE_2113123910117).

Strategy (expert-parallel + combine-weight-aware mixed precision):
  - Host: router softmax -> top-2 -> normalized combine weights. Every
    top-1 (token, expert) pair and every top-2 pair with combine weight
    s2 >= TAU runs on device in fp16. Top-2 pairs with s2 < TAU (~52% of
    them) run on device in fp8e4m3 using DoubleRow matmuls (2x PE MAC
    rate); their output error (~6.6% rel) is attenuated by s2 < 0.38,
    keeping the global rel error ~1.6e-2 (< 2e-2 budget).
  - Cores are grouped in pairs; each pair owns two experts. Per core the
    NEFF runs 4 sub-batches: A1/A2 = fp16 FFN over 768 tokens for expert
    i/j, B1/B2 = fp8 FFN over 256 tokens for expert i/j. Each expert's
    fp16 tokens split across its pair's two A-slots (2x768 = 1536 cap),
    fp8 tokens across the B-slots (512 cap). Capacity overflow (~1.5% of
    pairs, chosen as the largest-s2 fp8 tokens) is computed on host in
    fp32 and added during unshard.
  - fp16 path: y = diag(s) ((silu(x Wg^T) * (x Wu^T)) Wd^T), fp16 matmul
    operands, fp32 PSUM.
  - fp8 path: weights pre-scaled by 8 and quantized to fp8e4m3; x
    quantized to fp8e4m3. DoubleRow matmuls with K=256 per instruction.
    silu applied with scale 1/8 on ACT (fp16 out), h quantized to fp8 by
    the DVE multiply, down-projection epilogue scale s/64 folds out the
    weight prescaling.

Self-contained: hardcodes all shapes from the problem spec.
"""

import os
import numpy as np

# recover automatically if a prior run left the NeuronCores wedged
os.environ.setdefault("NEURON_RT_RESET_CORES", "1")

D = 1024
FF = 2048
E = 8
TOPK = 2
NCORES = 8
ND = D // 128    # 8 contraction chunks of 128
NF = FF // 128   # 16 ff chunks of 128
NKP = ND // 2    # 4 k-pairs (256-deep DoubleRow contraction) for gate/up
NFP = NF // 2    # 8 k-pairs for the fp8 down projection
CAP16 = 1536     # fp16 tokens per core (2 sub-batches of 768)
CAP8 = 512       # fp8 tokens per core (2 sub-batches of 256)
ATT = 256        # fp16 token tile
TAU = 0.38       # top-2 combine-weight threshold for the fp8 path
SW = 8.0         # fp8 weight prescale; down epilogue folds 1/SW^2

# test-only knobs / results (harness never touches these)
LAST_RESULTS = None
_NC_CACHE = {}


def split_multi_waits(nc, mybir_mod):
    """This walrus build rejects any instruction carrying more than one
    sync wait ("Too many sync wait commands"). Hoist extra waits onto
    single-wait NOPs inserted just before the instruction on the same
    engine — semantically identical since engines execute in order."""
    n_split = 0
    for f in nc.m.functions:
        for blk in f.blocks:
            insts = blk.instructions
            newl = []
            changed = False
            for inst in insts:
                si = inst.sync_info
                if si is not None and len(si.on_wait) > 1:
                    waits = list(si.on_wait)
                    del si.on_wait[1:]
                    for j, w in enumerate(waits[1:]):
                        nop = mybir_mod.InstNoOp(
                            name=f"{inst.name}_w{j}",
                            engine=inst.engine,
                            ins=[],
                            outs=[],
                        )
                        nop.sync_info = mybir_mod.SyncInfo(on_wait=[w], on_update=[])
                        newl.append(nop)
                        n_split += 1
                    changed = True
                newl.append(inst)
            if changed:
                insts[:] = newl
    return n_split


def build_nc(caps=(CAP16, CAP8), repeat=1):
    """Build the per-core Bass program: two fp16 expert FFN sub-batches
    followed by two fp8 (DoubleRow) sub-batches. Same NEFF on all 8
    cores (SPMD); the expert identity lives entirely in the input maps.

    repeat>1 wraps the whole body (including weight loads) in a hardware
    loop — used only for benchmarking (dispatch overhead amortization)."""
    import contextlib

    import concourse.bass as bass
    import concourse.mybir as mybir
    import concourse.tile as tile

    dt = mybir.dt
    f32 = dt.float32
    f16 = dt.float16
    e4 = dt.float8e4
    AF = mybir.ActivationFunctionType
    DR = mybir.MatmulPerfMode.DoubleRow

    cap16, cap8 = caps
    a_sub = cap16 // 2
    b_sub = cap8 // 2
    na_t = a_sub // ATT       # fp16 token tiles per sub-batch
    ng16 = cap16 // 128       # combine-scale groups, fp16
    ng8 = cap8 // 128

    nc = bass.Bass()
    # fp16 tokens: x^T arranged [128, d-chunk, token]
    xt = nc.dram_tensor("xt", [128, ND, cap16], f16, kind="ExternalInput")
    # fp8 tokens, same layout
    x8 = nc.dram_tensor("x8", [128, ND, cap8], e4, kind="ExternalInput")
    # fp16 weights, one set per expert of the pair; gate/up f-chunk-major
    wg = nc.dram_tensor("wg", [2, NF * 128, ND, 128], f16, kind="ExternalInput")
    wu = nc.dram_tensor("wu", [2, NF * 128, ND, 128], f16, kind="ExternalInput")
    wd = nc.dram_tensor("wd", [2, FF, D], f16, kind="ExternalInput")
    # fp8 weights (pre-scaled by SW): [set, chunk*128(+p), kpair, ktile, feat]
    wg8 = nc.dram_tensor("wg8", [2, NF * 128, NKP, 2, 128], e4, kind="ExternalInput")
    wu8 = nc.dram_tensor("wu8", [2, NF * 128, NKP, 2, 128], e4, kind="ExternalInput")
    # fp8 down weights: [set, p(f within chunk), kpair, ktile, d]
    wd8 = nc.dram_tensor("wd8", [2, 128, NFP, 2, D], e4, kind="ExternalInput")
    sc = nc.dram_tensor("sc", [128, ng16], f32, kind="ExternalInput")
    sc8 = nc.dram_tensor("sc8", [128, ng8], f32, kind="ExternalInput")
    # outputs at fp16: partial rows are combined on host in fp32
    y = nc.dram_tensor("y", [cap16, D], f16, kind="ExternalOutput")
    y8 = nc.dram_tensor("y8", [cap8, D], f16, kind="ExternalOutput")

    with tile.TileContext(nc) as tc:
        with (
            tc.tile_pool(name="wpool", bufs=1) as wpool,
            tc.tile_pool(name="w8pool", bufs=1) as w8pool,
            tc.tile_pool(name="xpool", bufs=3) as xpool,
            tc.tile_pool(name="hpool", bufs=2) as hpool,
            tc.tile_pool(name="gpool", bufs=3) as gpool,
            tc.tile_pool(name="g8pool", bufs=2) as g8pool,
            tc.tile_pool(name="ypool", bufs=4) as ypool,
            tc.tile_pool(name="pg", bufs=2, space="PSUM") as pgpool,
            tc.tile_pool(name="pu", bufs=2, space="PSUM") as pupool,
            tc.tile_pool(name="po", bufs=4, space="PSUM") as popool,
        ):
            # PE warmup: matmuls on a zeroed scratch tile while the first
            # DMAs are in flight, so the p-state clock gate is already
            # ramping when real matmuls start. Outside the repeat loop.
            warm = wpool.tile([128, 128], f16, tag="warm")
            nc.gpsimd.memset(warm[:], 0)
            pwarm = pgpool.tile([128, 512], f32, tag="pg")
            for i in range(44):
                nc.tensor.matmul(pwarm[:, 0:128], warm[:], warm[:])
            rep_ctx = (
                tc.For_i(0, repeat, 1, hint_engines=(mybir.EngineType.PE,))
                if repeat > 1
                else contextlib.nullcontext()
            )
            rep_ctx.__enter__()

            # Resident fp8 token tensor; fp16 tokens stream in 256-tiles.
            x8_sb = wpool.tile([128, ND, cap8], e4, tag="x8")
            s_sb = wpool.tile([128, ng16], f32, tag="s")
            s8_sb = wpool.tile([128, ng8], f32, tag="s8")

            # ---------------- phase A: fp16, two sub-batches ----------------
            for s in range(2):
                # DMA issue order == compute-need order. First sub-batch:
                # token tiles interleaved with the first weight slices so
                # the first matmul waits only ~1MB of DMA.
                wg_sb = [None] * NF
                wu_sb = [None] * NF
                xt_t = [None] * na_t

                def load_xt(t):
                    off = s * a_sub + t * ATT
                    xti = xpool.tile([128, ND, ATT], f16, tag="xt")
                    nc.sync.dma_start(xti[:], xt[:, :, off : off + ATT])
                    xt_t[t] = xti

                for f in range(NF):
                    if f == 0:
                        load_xt(0)
                    tg = wpool.tile([128, ND, 128], f16, tag=f"wg{f}")
                    nc.sync.dma_start(tg[:], wg[s, f * 128 : (f + 1) * 128])
                    wg_sb[f] = tg
                    tu = wpool.tile([128, ND, 128], f16, tag=f"wu{f}")
                    nc.sync.dma_start(tu[:], wu[s, f * 128 : (f + 1) * 128])
                    wu_sb[f] = tu
                    # token tiles interleaved into the weight stream in
                    # compute-need order (tile t is consumed only after all
                    # 16 f-chunks of tile t-1)
                    if f == 7 and na_t > 1:
                        load_xt(1)
                    elif f == NF - 1:
                        if s == 0:
                            nc.sync.dma_start(s_sb[:], sc[:])
                        for t in range(2, na_t):
                            load_xt(t)
                wd_sb = []
                for f in range(NF):
                    t_ = wpool.tile([128, D], f16, tag=f"wd{f}")
                    nc.sync.dma_start(t_[:], wd[s, f * 128 : (f + 1) * 128, :])
                    wd_sb.append(t_)

                for t in range(na_t):
                    off = s * a_sub + t * ATT
                    # gate/up + SwiGLU -> h^T [f, tokens]
                    ht_t = []
                    for f in range(NF):
                        pg = pgpool.tile([128, 512], f32, tag="pg")
                        pu = pupool.tile([128, 512], f32, tag="pu")
                        for j in range(ND):
                            nc.tensor.matmul(
                                pg[:, 0:ATT],
                                wg_sb[f][:, j, :],
                                xt_t[t][:, j, :],
                                start=(j == 0),
                                stop=(j == ND - 1),
                            )
                        for j in range(ND):
                            nc.tensor.matmul(
                                pu[:, 0:ATT],
                                wu_sb[f][:, j, :],
                                xt_t[t][:, j, :],
                                start=(j == 0),
                                stop=(j == ND - 1),
                            )
                        sg = gpool.tile([128, ATT], f16, tag="sg")
                        nc.scalar.activation(sg[:], pg[:, 0:ATT], AF.Silu)
                        ht = hpool.tile([128, ATT], f16, tag=f"ht{f}")
                        nc.vector.tensor_mul(ht[:], sg[:], pu[:, 0:ATT])
                        ht_t.append(ht)
                    # down projection, scaled by combine weight per token
                    for k in range(ATT // 128):
                        g = off // 128 + k
                        po_h = []
                        for dh in range(2):
                            po = popool.tile(
                                [128, 512], f32, tag="po", name=f"po_{s}_{t}_{k}_{dh}"
                            )
                            po_h.append(po)
                        for f in range(NF):
                            lhs = ht_t[f][:, k * 128 : (k + 1) * 128]
                            for dh in range(2):
                                nc.tensor.matmul(
                                    po_h[dh][:],
                                    lhs,
                                    wd_sb[f][:, dh * 512 : (dh + 1) * 512],
                                    start=(f == 0),
                                    stop=(f == NF - 1),
                                )
                        # scale-by-combine-weight copies: dh=0 on ACT, dh=1
                        # on DVE so the two run concurrently
                        for dh in range(2):
                            yt = ypool.tile([128, 512], f16, tag="yt")
                            if dh == 0:
                                nc.scalar.activation(
                                    yt[:], po_h[dh][:], AF.Copy,
                                    scale=s_sb[:, g : g + 1],
                                )
                            else:
                                nc.vector.tensor_scalar_mul(
                                    yt[:], po_h[dh][:], s_sb[:, g : g + 1]
                                )
                            dge = nc.sync if dh == 0 else nc.scalar
                            dge.dma_start(
                                y[
                                    g * 128 : (g + 1) * 128,
                                    dh * 512 : (dh + 1) * 512,
                                ],
                                yt[:],
                            )

            # ---------------- phase B: fp8 DoubleRow, two sub-batches -------
            # All phase-B DMAs are issued up front (they queue behind phase
            # A's loads and run during phase A compute). Gate/up set-1 tags
            # release chunk-by-chunk as sub-batch 0 consumes them; wd8 gets
            # one tag per set so set 1 streams during sub-batch 0 compute.
            wg8_sb = [{}, {}]
            wu8_sb = [{}, {}]
            wd8_sb = [None, None]
            nc.sync.dma_start(x8_sb[:], x8[:])
            nc.sync.dma_start(s8_sb[:], sc8[:])
            for c in range(NF):
                t8g = w8pool.tile([128, NKP, 2, 128], e4, tag=f"wg8_{c}")
                nc.sync.dma_start(t8g[:], wg8[0, c * 128 : (c + 1) * 128])
                wg8_sb[0][c] = t8g
                t8u = w8pool.tile([128, NKP, 2, 128], e4, tag=f"wu8_{c}")
                nc.sync.dma_start(t8u[:], wu8[0, c * 128 : (c + 1) * 128])
                wu8_sb[0][c] = t8u
            t8d = w8pool.tile([128, NFP, 2, D], e4, tag="wd8_0")
            nc.sync.dma_start(t8d[:], wd8[0])
            wd8_sb[0] = t8d

            for s in range(2):
                boff = s * b_sub
                ht8 = {}
                for cp in range(NF // 2):
                    # two 128-feature chunks share one [128,512] psum bank:
                    # only the chain's first matmul uses start=True — it
                    # zero-marks the whole 2KB bank, so the second chunk's
                    # chain accumulates onto pending-zero bytes
                    pg = pgpool.tile([128, 512], f32, tag="pg")
                    pu = pupool.tile([128, 512], f32, tag="pu")
                    for q in range(2):
                        c = 2 * cp + q
                        for j in range(NKP):
                            nc.tensor.matmul(
                                pg[:, q * ATT : (q + 1) * ATT],
                                wg8_sb[s][c][:, j, :, :],
                                x8_sb[:, 2 * j : 2 * j + 2, boff : boff + b_sub],
                                start=(q == 0 and j == 0),
                                stop=(q == 1 and j == NKP - 1),
                                perf_mode=DR,
                                skip_group_check=True,
                            )
                    for q in range(2):
                        c = 2 * cp + q
                        for j in range(NKP):
                            nc.tensor.matmul(
                                pu[:, q * ATT : (q + 1) * ATT],
                                wu8_sb[s][c][:, j, :, :],
                                x8_sb[:, 2 * j : 2 * j + 2, boff : boff + b_sub],
                                start=(q == 0 and j == 0),
                                stop=(q == 1 and j == NKP - 1),
                                perf_mode=DR,
                                skip_group_check=True,
                            )
                    sg = g8pool.tile([128, 512], f16, tag="sg8")
                    nc.scalar.activation(sg[:], pg[:], AF.Silu, scale=1.0 / SW)
                    hj = hpool.tile([128, 2, b_sub], e4, tag=f"ht8_{cp}")
                    ht8[cp] = hj
                    nc.vector.tensor_mul(hj[:], sg[:], pu[:])
                if s == 0:
                    # set-1 weight DMAs: emitted after sub-batch 0's gate/up
                    # reads are traced (tag reuse is then safe) so they
                    # stream during sub-batch 0's down sweep
                    for c in range(NF):
                        t8g = w8pool.tile([128, NKP, 2, 128], e4, tag=f"wg8_{c}")
                        nc.sync.dma_start(t8g[:], wg8[1, c * 128 : (c + 1) * 128])
                        wg8_sb[1][c] = t8g
                        t8u = w8pool.tile([128, NKP, 2, 128], e4, tag=f"wu8_{c}")
                        nc.sync.dma_start(t8u[:], wu8[1, c * 128 : (c + 1) * 128])
                        wu8_sb[1][c] = t8u
                    t8d = w8pool.tile([128, NFP, 2, D], e4, tag="wd8_1")
                    nc.sync.dma_start(t8d[:], wd8[1])
                    wd8_sb[1] = t8d
                # down projection: token groups of 128, d-halves of 512; the
                # two 256-wide chains of a half share one psum bank (start
                # only on the first) and one stationary load per j
                for k in range(b_sub // 128):
                    g = s * (b_sub // 128) + k
                    for dh in range(2):
                        po = popool.tile(
                            [128, 512], f32, tag="po", name=f"po8_{s}_{k}_{dh}"
                        )
                        for j in range(NFP):
                            for q in range(2):
                                nc.tensor.matmul(
                                    po[:, q * 256 : (q + 1) * 256],
                                    ht8[j][:, :, k * 128 : (k + 1) * 128],
                                    wd8_sb[s][
                                        :, j, :,
                                        dh * 512 + q * 256 : dh * 512 + (q + 1) * 256,
                                    ],
                                    start=(j == 0 and q == 0),
                                    stop=(j == NFP - 1 and q == 1),
                                    perf_mode=DR,
                                    skip_group_check=True,
                                )
                        yt = ypool.tile([128, 512], f16, tag="yt")
                        if dh == 0:
                            nc.scalar.activation(
                                yt[:], po[:], AF.Copy, scale=s8_sb[:, g : g + 1]
                            )
                        else:
                            nc.vector.tensor_scalar_mul(
                                yt[:], po[:], s8_sb[:, g : g + 1]
                            )
                        dge = nc.sync if dh == 0 else nc.scalar
                        dge.dma_start(
                            y8[
                                g * 128 : (g + 1) * 128,
                                dh * 512 : (dh + 1) * 512,
                            ],
                            yt[:],
                        )
            rep_ctx.__exit__(None, None, None)
    split_multi_waits(nc, mybir)
    return nc


def _get_nc(caps=(CAP16, CAP8)):
    key = caps
    if key not in _NC_CACHE:
        _NC_CACHE[key] = build_nc(caps)
    return _NC_CACHE[key]


def _route(xf, Wr):
    """fp32 softmax + top-2 + normalized combine weights, matching the
    jax reference (ties broken toward lower expert index)."""
    logits = xf @ Wr.astype(np.float32).T
    m = logits.max(-1, keepdims=True)
    ex = np.exp(logits - m)
    p = ex / ex.sum(-1, keepdims=True)
    top2 = np.argsort(-p, axis=-1, kind="stable")[:, :TOPK]
    n = xf.shape[0]
    p1 = p[np.arange(n), top2[:, 0]]
    p2 = p[np.arange(n), top2[:, 1]]
    denom = (p1 + p2) + np.float32(1e-8)
    return top2, p1 / denom, p2 / denom


def _pack_w16(W, mmnp):
    """Gate/up fp16 weight packing: [NF*128, ND, 128] f-chunk-major."""
    WT = W.T.reshape(ND, 128, NF, 128)
    return np.ascontiguousarray(
        WT.transpose(2, 1, 0, 3).reshape(NF * 128, ND, 128)
    ).astype(mmnp)


def _pack_w8_gu(W, e4np):
    """fp8 gate/up packing: [NF*128, NKP, 2, 128] where
    [c*128+p, j, i, m] = SW*W[c*128+m, (2j+i)*128+p]."""
    t = (W * SW).reshape(NF, 128, NKP, 2, 128)  # [c, m, j, i, p]
    return np.ascontiguousarray(t.transpose(0, 4, 2, 3, 1)).reshape(
        NF * 128, NKP, 2, 128
    ).astype(e4np)


def _pack_w8_d(Wde, e4np):
    """fp8 down packing: [128, NFP, 2, D] where
    [p, j, i, d] = SW*Wd[d, (2j+i)*128+p]."""
    t = (Wde * SW).reshape(D, NFP, 2, 128)  # [d, j, i, p]
    return np.ascontiguousarray(t.transpose(3, 1, 2, 0)).astype(e4np)


def _prep_maps(inputs, caps=(CAP16, CAP8)):
    """Route + build per-core input maps. Returns
    (in_maps, seg16, seg8, overflow, xf) where seg16/seg8 map core ->
    token index array for the y/y8 outputs and overflow is a list of
    (expert, token_idx_array, scale_array) computed on host."""
    import ml_dtypes

    cap16, cap8 = caps
    a_sub = cap16 // 2
    b_sub = cap8 // 2
    x = np.asarray(inputs["x"])
    Wr = np.asarray(inputs["Wr"])
    Wg = np.asarray(inputs["Wg"])
    Wu = np.asarray(inputs["Wu"])
    Wd = np.asarray(inputs["Wd"])
    xf = x.reshape(-1, D).astype(np.float32, copy=False)

    top2, s1, s2 = _route(xf, Wr)

    f16np = np.dtype(np.float16)
    e4np = np.dtype(ml_dtypes.float8_e4m3)
    xf_16 = xf.astype(f16np)
    xf_8 = xf.astype(e4np)

    # per-expert token lists
    idx16_e, sc16_e, idx8_e, sc8_e = [], [], [], []
    overflow = []
    n = xf.shape[0]
    for e in range(E):
        i_top1 = np.nonzero(top2[:, 0] == e)[0]
        s_top1 = s1[i_top1]
        i_top2 = np.nonzero(top2[:, 1] == e)[0]
        s_top2 = s2[i_top2]
        hi = s_top2 >= TAU
        i16 = np.concatenate([i_top1, i_top2[hi]])
        sc16v = np.concatenate([s_top1, s_top2[hi]]).astype(np.float32)
        # fp8 candidates sorted ascending s2 so capacity spill takes the
        # largest-s2 tokens (computed exactly on host -> least fp8 error)
        i8 = i_top2[~hi]
        s8v = s_top2[~hi].astype(np.float32)
        o = np.argsort(s8v, kind="stable")
        i8, s8v = i8[o], s8v[o]
        if len(i16) > cap16:
            overflow.append((e, i16[cap16:], sc16v[cap16:]))
            i16, sc16v = i16[:cap16], sc16v[:cap16]
        if len(i8) > cap8:
            overflow.append((e, i8[cap8:], s8v[cap8:]))
            i8, s8v = i8[:cap8], s8v[:cap8]
        idx16_e.append(i16)
        sc16_e.append(sc16v)
        idx8_e.append(i8)
        sc8_e.append(s8v)

    # pair experts to balance per-core cycles (384/pair fp16, 192 fp8)
    cyc = [384 * len(idx16_e[e]) + 192 * len(idx8_e[e]) for e in range(E)]
    order = np.argsort(np.asarray(cyc))
    pairs = [(int(order[7 - k]), int(order[k])) for k in range(E // 2)]

    in_maps = [None] * NCORES
    seg16 = [None] * NCORES
    seg8 = [None] * NCORES
    for k, (ea, eb) in enumerate(pairs):
        wgs = np.stack([_pack_w16(Wg[ea], f16np), _pack_w16(Wg[eb], f16np)])
        wus = np.stack([_pack_w16(Wu[ea], f16np), _pack_w16(Wu[eb], f16np)])
        wds = np.stack(
            [
                np.ascontiguousarray(Wd[ea].T).astype(f16np),
                np.ascontiguousarray(Wd[eb].T).astype(f16np),
            ]
        )
        wg8s = np.stack([_pack_w8_gu(Wg[ea], e4np), _pack_w8_gu(Wg[eb], e4np)])
        wu8s = np.stack([_pack_w8_gu(Wu[ea], e4np), _pack_w8_gu(Wu[eb], e4np)])
        wd8s = np.stack([_pack_w8_d(Wd[ea], e4np), _pack_w8_d(Wd[eb], e4np)])
        for half in range(2):
            core = 2 * k + half
            xt3 = np.zeros((128, ND, cap16), dtype=f16np)
            x83 = np.zeros((128, ND, cap8), dtype=e4np)
            scp = np.zeros(cap16, dtype=np.float32)
            sc8p = np.zeros(cap8, dtype=np.float32)
            t16 = []
            t8 = []
            for s, e_ in enumerate((ea, eb)):
                i16, v16 = idx16_e[e_], sc16_e[e_]
                nhalf = (len(i16) + 1) // 2
                seg = i16[:nhalf] if half == 0 else i16[nhalf:]
                vseg = v16[:nhalf] if half == 0 else v16[nhalf:]
                assert len(seg) <= a_sub
                off = s * a_sub
                xt3[:, :, off : off + len(seg)] = (
                    xf_16[seg].T.reshape(ND, 128, len(seg)).transpose(1, 0, 2)
                )
                scp[off : off + len(seg)] = vseg
                t16.append(seg)
                i8, v8 = idx8_e[e_], sc8_e[e_]
                nhalf8 = (len(i8) + 1) // 2
                seg8_ = i8[:nhalf8] if half == 0 else i8[nhalf8:]
                vseg8 = v8[:nhalf8] if half == 0 else v8[nhalf8:]
                assert len(seg8_) <= b_sub
                off8 = s * b_sub
                x83[:, :, off8 : off8 + len(seg8_)] = (
                    xf_8[seg8_]
                    .astype(e4np)
                    .T.reshape(ND, 128, len(seg8_))
                    .transpose(1, 0, 2)
                )
                sc8p[off8 : off8 + len(seg8_)] = vseg8 / np.float32(SW * SW)
                t8.append(seg8_)
            in_maps[core] = {
                "xt": xt3,
                "x8": x83,
                "wg": wgs,
                "wu": wus,
                "wd": wds,
                "wg8": wg8s,
                "wu8": wu8s,
                "wd8": wd8s,
                "sc": np.ascontiguousarray(
                    scp.reshape(cap16 // 128, 128).T
                ),
                "sc8": np.ascontiguousarray(
                    sc8p.reshape(cap8 // 128, 128).T
                ),
            }
            seg16[core] = t16
            seg8[core] = t8
    return in_maps, seg16, seg8, overflow, xf


def kernel(**inputs):
    global LAST_RESULTS
    from concourse.bass_utils import run_bass_kernel_spmd

    x = np.asarray(inputs["x"])
    B, T, _ = x.shape
    caps = (CAP16, CAP8)
    in_maps, seg16, seg8, overflow, xf = _prep_maps(inputs, caps)
    n_tok = xf.shape[0]
    a_sub, b_sub = caps[0] // 2, caps[1] // 2

    nc = _get_nc(caps)
    try:
        res = run_bass_kernel_spmd(nc, in_maps, list(range(NCORES)))
    except Exception:
        # A previously wedged NeuronCore fails the first execute attempt
        # (NRT_EXEC_UNIT_UNRECOVERABLE); resetting the PJRT backend and
        # retrying once recovers (cores reset via NEURON_RT_RESET_CORES).
        import jax
        import jax.extend as jex

        jax.clear_caches()
        try:
            jex.backend.clear_backends()
        except Exception:
            pass
        res = run_bass_kernel_spmd(nc, in_maps, list(range(NCORES)))
    LAST_RESULTS = res

    out = np.zeros((n_tok, D), dtype=np.float32)
    for core in range(NCORES):
        y16 = np.asarray(res.results[core]["y"]).astype(np.float32)
        y8v = np.asarray(res.results[core]["y8"]).astype(np.float32)
        for s in range(2):
            seg = seg16[core][s]
            out[seg] += y16[s * a_sub : s * a_sub + len(seg)]
            sg8 = seg8[core][s]
            out[sg8] += y8v[s * b_sub : s * b_sub + len(sg8)]
    # host-side capacity-overflow spill (fp32, exact)
    if overflow:
        Wg = np.asarray(inputs["Wg"], dtype=np.float32)
        Wu = np.asarray(inputs["Wu"], dtype=np.float32)
        Wd = np.asarray(inputs["Wd"], dtype=np.float32)
        for e, oidx, osc in overflow:
            xo = xf[oidx]
            g = xo @ Wg[e].T
            u = xo @ Wu[e].T
            h = (g / (1.0 + np.exp(-g))) * u
            out[oidx] += osc[:, None] * (h @ Wd[e].T)
    return out.reshape(B, T, D).astype(x.dtype, copy=False)
